# revision 6
# baseline (speedup 1.0000x reference)
"""Multi-head self-attention (B=2, N=4096, D=768, H=12, dh=64) on 8 trn2 NeuronCores.

Sharding: core c handles batch b=c//4 and heads 3*(c%4)..3*(c%4)+2 (head-parallel
attention), then an AllGather redistributes head-outputs so each core projects its
own 192 output dims with a col-slice of w_out (column-parallel output projection).

Per-core pipeline:
  1. qkv projection from host-transposed xT (fp32r matmuls), producing qT/kT
     (d-on-partition, bf16, head-pair packed for PE row-tiling) and v (token-on-
     partition bf16, via PE transpose), k pre-scaled by softmax_scale/8.  Five
     m-chunks (not six): head-2 q/k share one chunk and are duplicated into both
     row-halves afterwards by SBUF-to-SBUF DMA.
  2. flash-style attention per 512-query chunk: row-tiled QK^T (bf16) -> exp
     (split between ACT table exp and a custom 2-op DVE polynomial exp, bf16
     outputs) -> PV accumulation (bf16) with an appended ones-column producing
     the softmax denominator -> normalize via 1/l = exp(-ln l) on ACT, row
     broadcast on GPSIMD, multiply on DVE.
  3. AllGather over each batch's 4 cores + output projection + bias, with each
     gathered chunk's projection interleaved into the attention loop.
"""
import sys

sys.path.insert(0, "/opt/trn_rl_repo")

import numpy as np

import concourse.bass as bass
import concourse.mybir as mybir
import concourse.tile as tile
import concourse.bacc as bacc
from concourse.masks import make_identity

N_CORES = 8
B, N, D, H, DH = 2, 4096, 768, 12, 64
HPC = 3            # heads per core
SCALE = D ** -0.5
F32 = mybir.dt.float32
F32R = mybir.dt.float32r
AF = mybir.ActivationFunctionType
BF16 = mybir.dt.bfloat16
IC = 512           # query chunk (i-chunk) size
NIC = N // IC      # 8 i-chunks
NTAU = N // IC     # token chunks for projection (512 wide)
NM = 5             # m-chunks: [q0|q1], [k0|k1], [v0|v1], [q2|k2], [v2|junk]
VW = 195           # v_sb row stride: [v0|1|v1|1|v2|1] = 3*65
DVE_EXP_MOD = 3    # 1 of every 3 exp batches goes to the DVE poly-exp
DVE_EXP_CNT = 1


# ---------------------------------------------------------------- custom DVE exp
def _register_exp_ops():
    """exp(8u) as two DVE ops: EXP_P4_ANT = taylor4(u); EXP_SQ8_ANT = x^8."""
    import concourse.dve_ops as dve_ops
    from concourse.dve_ops import DveOp, OPS, CUSTOM_DVE_SPECS, _SUB_OPCODE_FOR_NAME
    from concourse.dve_spec import Spec, Src0, C0, C1, C2, One, sq, lower
    from concourse.dve_uop import DveOpSpec

    if "EXP_P4_ANT" in _SUB_OPCODE_FOR_NAME:
        return dve_ops.EXP_P4_ANT, dve_ops.EXP_SQ8_ANT

    u = Src0
    p4 = ((((u * C0) + C1) * u + C2) * u + One) * u + One  # c0=1/24 c1=1/6 c2=1/2
    spec_p4 = Spec(
        body=p4,
        reference=lambda in0, in1, s0, s1, imm2: (
            (((in0 * s0 + s1) * in0 + imm2) * in0 + 1.0) * in0 + 1.0
        ),
    )
    spec_sq8 = Spec(
        body=sq(sq(sq(Src0))),
        reference=lambda in0, in1, s0, s1, imm2: in0 ** 8,
    )

    def _mk(name, spec):
        opcode = max(_SUB_OPCODE_FOR_NAME.values()) + 1
        _SUB_OPCODE_FOR_NAME[name] = opcode
        shas = {}
        for ver in ("v3", "v4"):
            s = DveOpSpec(
                name=name, opcode=opcode, uops=lower(spec, ver=ver), rd1_en=False
            )
            shas[ver] = s.sha(ver)
        op = DveOp(name, spec, subdim=False, uops_sha=shas)
        OPS.append(op)
        CUSTOM_DVE_SPECS[name] = spec
        setattr(dve_ops, name, op)
        return op

    p4_op = _mk("EXP_P4_ANT", spec_p4)
    sq8_op = _mk("EXP_SQ8_ANT", spec_sq8)
    return p4_op, sq8_op


# ---------------------------------------------------------------- program build
_PROG_CACHE = {}


def build_program(use_dve_exp=True):
    key = ("prog", use_dve_exp)
    if key in _PROG_CACHE:
        return _PROG_CACHE[key]
    p4_op, sq8_op = _register_exp_ops()

    nc = bacc.Bacc("TRN2", target_bir_lowering=False, debug=False, num_devices=N_CORES)

    xT = nc.dram_tensor("xT", [D, N], F32, kind="ExternalInput").ap()
    wqkv = nc.dram_tensor("wqkv", [D, NM * 128], F32, kind="ExternalInput").ap()
    bqkv = nc.dram_tensor("bqkv", [NM, 128], F32, kind="ExternalInput").ap()
    wout = nc.dram_tensor("wout", [D, 192], F32, kind="ExternalInput").ap()
    bout = nc.dram_tensor("bout", [2, 128], F32, kind="ExternalInput").ap()
    y = nc.dram_tensor("y", [HPC * DH, N], F32, kind="ExternalOutput").ap()

    xT_r = xT.bitcast(F32R)
    wqkv_r = wqkv.bitcast(F32R)

    with tile.TileContext(nc, trace_sim=False) as tc:
        with (
            tc.tile_pool(name="consts", bufs=1) as consts,
            tc.tile_pool(name="persist", bufs=1) as persist,
            tc.tile_pool(name="otp", bufs=3) as otp,
            tc.tile_pool(name="spsum", bufs=2, space="PSUM") as spsum,
            tc.tile_pool(name="opsum", bufs=1, space="PSUM") as opsum,
            tc.tile_pool(name="dram", bufs=1, space="DRAM") as dram,
        ):
            # ---------------- constants
            ident_f = consts.tile([128, 128], F32)
            make_identity(nc, ident_f[:])
            ident = consts.tile([128, 128], BF16)
            nc.scalar.copy(ident[:], ident_f[:])

            wq_sb = consts.tile([128, 6 * NM * 128], F32R)  # 6 f-chunks of [128,640]
            for fc in range(6):
                nc.sync.dma_start(
                    out=wq_sb[:, fc * NM * 128 : (fc + 1) * NM * 128],
                    in_=wqkv_r[fc * 128 : (fc + 1) * 128, :],
                )
            bq_sb = consts.tile([128, NM], F32)  # per-m-chunk bias columns
            for m in range(NM):
                nc.sync.dma_start(
                    out=bq_sb[:, m : m + 1],
                    in_=bqkv[m : m + 1, :].rearrange("a p -> p a"),
                )
            # phase-3 weights (loaded early, used late)
            wo_sb = consts.tile([128, 6 * 192], BF16)  # w_out col-slice, 6 d-chunks
            for dc in range(6):
                nc.gpsimd.dma_start(
                    out=wo_sb[:, dc * 192 : (dc + 1) * 192],
                    in_=wout[dc * 128 : (dc + 1) * 128, :],
                )
            bo_sb = consts.tile([128, 2], F32)
            for m in range(2):
                nc.sync.dma_start(
                    out=bo_sb[:, m : m + 1],
                    in_=bout[m : m + 1, :].rearrange("a p -> p a"),
                )

            # ---------------- persistent activations (bf16)
            qT01 = persist.tile([128, N], BF16)  # rows 0:64 head0 qT, 64:128 head1
            qT2 = persist.tile([128, N], BF16)   # head2 qT duplicated in both halves
            kT01 = persist.tile([128, N], BF16)  # scaled kT, heads 0/1
            kT2 = persist.tile([128, N], BF16)   # scaled kT head2, duplicated
            v_sb = persist.tile([128, 32 * VW], BF16)  # [tok128][v0|1|v1|1|v2|1]/j-tile

            # ones columns of v_sb (positions 64,129,194 of each 195 block)
            ones32 = consts.tile([128, 32], BF16)
            nc.vector.memset(ones32[:], 1.0)
            v_view = v_sb[:].rearrange("p (t c) -> p t c", c=VW)
            for col in (64, 129, 194):
                nc.vector.tensor_copy(
                    v_view[:, :, col : col + 1],
                    ones32[:].rearrange("p (a b) -> p a b", b=1),
                )

            # AllGather eighths: in [192, 512] -> out [768, 512] (rank-major rows)
            oT_q = [dram.tile([HPC * DH, IC], BF16, name=f"oT_q{i}") for i in range(8)]
            ag_q = [dram.tile([D, IC], BF16, name=f"ag_q{i}") for i in range(8)]

            with tc.tile_pool(name="work", bufs=2) as work, \
                 tc.tile_pool(name="ptp", bufs=3) as ptp:
                # ---------------- phase 1: qkv projection
                for tau in range(NTAU):
                    ts = slice(tau * IC, (tau + 1) * IC)
                    xts = []
                    for fc in range(6):
                        xt = work.tile([128, IC], F32R, tag="xt", bufs=7)
                        nc.sync.dma_start(
                            out=xt[:], in_=xT_r[fc * 128 : (fc + 1) * 128, ts]
                        )
                        xts.append(xt)
                    vt01 = work.tile([128, IC], BF16, tag="vt01")
                    vt2 = work.tile([64, IC], BF16, tag="vt2")
                    for m in range(NM):
                        pp = spsum.tile([128, 1024], F32, tag="sp", bufs=2)
                        for fc in range(6):
                            nc.tensor.matmul(
                                pp[:, 0:IC],
                                wq_sb[:, fc * NM * 128 + m * 128
                                      : fc * NM * 128 + (m + 1) * 128],
                                xts[fc][:],
                                start=(fc == 0),
                                stop=(fc == 5),
                            )
                        if m == 0:
                            nc.scalar.activation(
                                qT01[:, ts], pp[:, 0:IC], AF.Identity,
                                bias=bq_sb[:, 0:1],
                            )
                        elif m == 1:
                            nc.scalar.activation(
                                kT01[:, ts], pp[:, 0:IC], AF.Identity,
                                bias=bq_sb[:, 1:2],
                            )
                        elif m == 2:
                            nc.scalar.activation(
                                vt01[:], pp[:, 0:IC], AF.Identity,
                                bias=bq_sb[:, 2:3],
                            )
                        elif m == 3:
                            # [q2 | k2]: split halves, partition-aligned writes
                            nc.scalar.activation(
                                qT2[0:64, ts], pp[0:64, 0:IC], AF.Identity,
                                bias=bq_sb[0:64, 3:4],
                            )
                            nc.scalar.activation(
                                kT2[64:128, ts], pp[64:128, 0:IC], AF.Identity,
                                bias=bq_sb[64:128, 3:4],
                            )
                        else:
                            nc.scalar.activation(
                                vt2[:], pp[0:64, 0:IC], AF.Identity,
                                bias=bq_sb[0:64, 4:5],
                            )
                    # transpose v into token-major layout (DVE does the copies)
                    for t in range(4):
                        jt = 4 * tau + t
                        base = jt * VW
                        pv = spsum.tile([128, 2048], BF16, tag="sp", bufs=2)
                        nc.tensor.transpose(
                            pv[:, 0:128], vt01[:, t * 128 : (t + 1) * 128], ident[:]
                        )
                        nc.vector.tensor_copy(
                            v_sb[:, base : base + 130].rearrange(
                                "p (a b) -> p a b", b=65
                            )[:, :, 0:64],
                            pv[:, 0:128].rearrange("p (a b) -> p a b", b=64),
                        )
                        pv2 = spsum.tile([128, 2048], BF16, tag="sp", bufs=2)
                        nc.tensor.transpose(
                            pv2[:, 0:64],
                            vt2[:, t * 128 : (t + 1) * 128],
                            ident[0:64, 0:64],
                        )
                        nc.vector.tensor_copy(
                            v_sb[:, base + 130 : base + 194], pv2[:, 0:64]
                        )

                # duplicate head-2 q/k into the other row-half (SBUF->SBUF DMA)
                nc.sync.dma_start(out=qT2[64:128, :], in_=qT2[0:64, :])
                nc.sync.dma_start(out=kT2[0:64, :], in_=kT2[64:128, :])

                # ---------------- phase 2: attention
                exp_batch_idx = [0]

                def exp_batch(sp):
                    """exp(8u) on a [128,1024] psum batch -> bf16 SBUF tile."""
                    i = exp_batch_idx[0]
                    exp_batch_idx[0] += 1
                    pt = ptp.tile([128, 1024], BF16, tag="pt", bufs=5)
                    if use_dve_exp and (i % DVE_EXP_MOD) < DVE_EXP_CNT:
                        tmp = ptp.tile([128, 1024], F32, tag="exptmp", bufs=3)
                        nc.vector._custom_dve(
                            p4_op, out=tmp[:], in0=sp[:],
                            s0=1.0 / 24, s1=1.0 / 6, imm2=0.5,
                        )
                        nc.vector._custom_dve(sq8_op, out=pt[:], in0=tmp[:])
                    else:
                        nc.scalar.activation(pt[:], sp[:], AF.Exp, scale=8.0)
                    return pt

                def norm_and_store(po, ic, h):
                    """normalize [65,512] psum (row 64 = l) and DMA to oT_dram."""
                    lt = otp.tile([1, IC], F32, tag="lt")
                    rr = otp.tile([1, IC], F32, tag="rr")
                    with nc.allow_low_precision(reason="softmax denom recip"):
                        nc.scalar.activation(lt[:], po[64:65, :], AF.Ln)
                        nc.scalar.activation(rr[:], lt[:], AF.Exp, scale=-1.0)
                    rb = otp.tile([64, IC], F32, tag="rb")
                    nc.gpsimd.partition_broadcast(rb[:], rr[:], channels=64)
                    ot = otp.tile([64, IC], BF16, tag="ot")
                    nc.vector.tensor_tensor(
                        out=ot[:], in0=po[0:64, :], in1=rb[:],
                        op=mybir.AluOpType.mult,
                    )
                    nc.sync.dma_start(
                        out=oT_q[ic][64 * h : 64 * h + 64, :], in_=ot[:]
                    )

                def project(qtr):
                    """output projection of gathered chunk qtr (col-parallel)."""
                    agr = ag_q[qtr][:]
                    ogs = []
                    for dc in range(6):
                        og = work.tile([128, IC], BF16, tag="og", bufs=12)
                        nc.sync.dma_start(
                            out=og[:], in_=agr[dc * 128 : (dc + 1) * 128, :]
                        )
                        ogs.append(og)
                    for ec, (elo, ew) in enumerate(((0, 128), (128, 64))):
                        py = spsum.tile([128, 1024], F32, tag="sp", bufs=2)
                        for dc in range(6):
                            nc.tensor.matmul(
                                py[0:ew, 0:IC],
                                wo_sb[:, dc * 192 + elo : dc * 192 + elo + ew],
                                ogs[dc][:],
                                start=(dc == 0), stop=(dc == 5),
                            )
                        ysb = work.tile([128, IC], F32, tag="ysb", bufs=3)
                        nc.scalar.activation(
                            ysb[0:ew, :], py[0:ew, 0:IC], AF.Identity,
                            bias=bo_sb[0:ew, ec : ec + 1],
                        )
                        nc.sync.dma_start(
                            out=y[elo : elo + ew, qtr * IC : (qtr + 1) * IC],
                            in_=ysb[0:ew, :],
                        )

                for ic in range(NIC):
                    isl = slice(ic * IC, (ic + 1) * IC)
                    # --- heads 0,1 (row-tiled pair over the same key tile)
                    po0 = opsum.tile([128, IC], F32, tag="po0", bufs=1)
                    po1 = opsum.tile([128, IC], F32, tag="po1", bufs=1)
                    for jc in range(32):
                        sp = spsum.tile([128, 1024], F32, tag="sp", bufs=2)
                        nc.tensor.matmul(
                            sp[:, 0:512],
                            kT01[0:64, jc * 128 : (jc + 1) * 128],
                            qT01[0:64, isl],
                            start=True, stop=True, tile_position=(0, 0),
                        )
                        nc.tensor.matmul(
                            sp[:, 512:1024],
                            kT01[64:128, jc * 128 : (jc + 1) * 128],
                            qT01[64:128, isl],
                            start=True, stop=True, tile_position=(64, 0),
                        )
                        pt = exp_batch(sp)
                        vbase = jc * VW
                        nc.tensor.matmul(
                            po0[0:65, :], v_sb[:, vbase : vbase + 65], pt[:, 0:512],
                            start=(jc == 0), stop=(jc == 31),
                        )
                        nc.tensor.matmul(
                            po1[0:65, :], v_sb[:, vbase + 65 : vbase + 130],
                            pt[:, 512:1024],
                            start=(jc == 0), stop=(jc == 31),
                        )
                    # --- head 2 (row-tiled pair over adjacent key tiles)
                    po2 = opsum.tile([128, IC], F32, tag="po2", bufs=1)
                    for t in range(16):
                        if t == 8:
                            norm_and_store(po0, ic, 0)
                            norm_and_store(po1, ic, 1)
                        sp = spsum.tile([128, 1024], F32, tag="sp", bufs=2)
                        nc.tensor.matmul(
                            sp[:, 0:512],
                            kT2[0:64, (2 * t) * 128 : (2 * t + 1) * 128],
                            qT2[0:64, isl],
                            start=True, stop=True, tile_position=(0, 0),
                        )
                        nc.tensor.matmul(
                            sp[:, 512:1024],
                            kT2[64:128, (2 * t + 1) * 128 : (2 * t + 2) * 128],
                            qT2[64:128, isl],
                            start=True, stop=True, tile_position=(64, 0),
                        )
                        pt = exp_batch(sp)
                        for s in range(2):
                            jc = 2 * t + s
                            vbase = jc * VW
                            nc.tensor.matmul(
                                po2[0:65, :],
                                v_sb[:, vbase + 130 : vbase + 195],
                                pt[:, s * 512 : (s + 1) * 512],
                                start=(jc == 0), stop=(jc == 31),
                            )
                    norm_and_store(po2, ic, 2)
                    nc.gpsimd.collective_compute(
                        "AllGather",
                        mybir.AluOpType.bypass,
                        replica_groups=[[0, 1, 2, 3], [4, 5, 6, 7]],
                        ins=[oT_q[ic][:]],
                        outs=[ag_q[ic][:]],
                    )
                    # interleave output projection of the previous chunk
                    if ic >= 1:
                        project(ic - 1)
                project(NIC - 1)

    nc.compile()
    _PROG_CACHE[key] = nc
    return nc


# ---------------------------------------------------------------- host wrapper
def make_in_maps(x, w_qkv, b_qkv, w_out, b_out):
    """Build the 8 per-core input dicts from full inputs."""
    in_maps = []
    xTb = [np.ascontiguousarray(x[b].T) for b in range(B)]  # [768, 4096]
    kscale = np.float32(SCALE / 8.0)
    for c in range(N_CORES):
        b = c // 4
        hs = HPC * (c % 4)

        def sect(kind, h):  # q=0,k=1,v=2
            lo = kind * (H * DH) + h * DH
            return w_qkv[:, lo : lo + DH], b_qkv[lo : lo + DH]

        q0, bq0 = sect(0, hs); q1, bq1 = sect(0, hs + 1); q2, bq2 = sect(0, hs + 2)
        k0, bk0 = sect(1, hs); k1, bk1 = sect(1, hs + 1); k2, bk2 = sect(1, hs + 2)
        v0, bv0 = sect(2, hs); v1, bv1 = sect(2, hs + 1); v2, bv2 = sect(2, hs + 2)
        z = np.zeros_like(q2); bz = np.zeros_like(bq2)
        # m-chunks: [q0|q1], [k0|k1]*s, [v0|v1], [q2|k2]*s, [v2|0]
        cols = np.concatenate(
            [q0, q1, k0 * kscale, k1 * kscale, v0, v1,
             q2, k2 * kscale, v2, z], axis=1).astype(np.float32)
        bias = np.concatenate(
            [bq0, bq1, bk0 * kscale, bk1 * kscale, bv0, bv1,
             bq2, bk2 * kscale, bv2, bz]).astype(np.float32)
        q = c % 4
        bo = np.zeros((2, 128), np.float32)
        bo[0, :] = b_out[192 * q : 192 * q + 128]
        bo[1, :64] = b_out[192 * q + 128 : 192 * q + 192]
        in_maps.append({
            "xT": xTb[b],
            "wqkv": np.ascontiguousarray(cols),
            "bqkv": np.ascontiguousarray(bias.reshape(NM, 128)),
            "wout": np.ascontiguousarray(
                w_out[:, 192 * q : 192 * (q + 1)].astype(np.float32)),
            "bout": bo,
        })
    return in_maps


def assemble_output(results):
    out = np.empty((B, N, D), dtype=np.float32)
    for c in range(N_CORES):
        b = c // 4
        q = c % 4
        out[b, :, 192 * q : 192 * (q + 1)] = results[c]["y"].T
    return out


def kernel(x, w_qkv, b_qkv, w_out, b_out):
    from concourse.bass_utils import run_bass_kernel_spmd

    x = np.asarray(x, dtype=np.float32)
    nc = build_program()
    in_maps = make_in_maps(
        x, np.asarray(w_qkv, np.float32), np.asarray(b_qkv, np.float32),
        np.asarray(w_out, np.float32), np.asarray(b_out, np.float32))
    res = run_bass_kernel_spmd(nc, in_maps, core_ids=list(range(N_CORES)))
    return assemble_output(res.results)


# revision 20
# speedup vs baseline: 1.4108x; 1.4108x over previous
"""Multi-head self-attention (B=2, N=4096, D=768, H=12, dh=64) on 8 trn2 NeuronCores.

Sharding: core c handles batch b=c//4 and heads 3*(c%4)..3*(c%4)+2 (head-parallel
attention), then an AllGather redistributes head-outputs so each core projects its
own 192 output dims with a col-slice of w_out (column-parallel output projection).

Per-core pipeline:
  1. qkv projection from host-transposed xT (fp32r matmuls), producing qT/kT
     (d-on-partition, bf16, head-pair packed for PE row-tiling) and v (token-on-
     partition bf16, via PE transpose), k pre-scaled by softmax_scale/8.  Five
     m-chunks (not six): head-2 q/k share one chunk and are duplicated into both
     row-halves afterwards by SBUF-to-SBUF DMA.
  2. flash-style attention per 512-query chunk: row-tiled QK^T (bf16) -> exp
     (split between ACT table exp and a custom 2-op DVE polynomial exp, bf16
     outputs) -> PV accumulation (bf16) with an appended ones-column producing
     the softmax denominator -> normalize via 1/l = exp(-ln l) on ACT, row
     broadcast on GPSIMD, multiply on DVE.
  3. AllGather over each batch's 4 cores + output projection + bias, with each
     gathered chunk's projection interleaved into the attention loop.
"""
import sys

sys.path.insert(0, "/opt/trn_rl_repo")

import numpy as np

import concourse.bass as bass
import concourse.mybir as mybir
import concourse.tile as tile
import concourse.bacc as bacc
from concourse.masks import make_identity

N_CORES = 8
B, N, D, H, DH = 2, 4096, 768, 12, 64
HPC = 3            # heads per core
SCALE = D ** -0.5
F32 = mybir.dt.float32
F32R = mybir.dt.float32r
AF = mybir.ActivationFunctionType
BF16 = mybir.dt.bfloat16
IC = 512           # query chunk (i-chunk) size
NIC = N // IC      # 8 i-chunks
NTAU = N // IC     # token chunks for projection (512 wide)
NM = 5             # m-chunks: [q0|q1], [k0|k1], [v0|v1], [q2|k2], [v2|junk]
VW = 195           # v_sb row stride: [v0|1|v1|1|v2|1] = 3*65
DVE_EXP_MOD = 3    # 1 of every 3 exp batches goes to the DVE poly-exp
DVE_EXP_CNT = 1


# ---------------------------------------------------------------- custom DVE exp
def _register_exp_ops():
    """exp(8u) as two DVE ops: EXP_P4_ANT = taylor4(u); EXP_SQ8_ANT = x^8."""
    import concourse.dve_ops as dve_ops
    from concourse.dve_ops import DveOp, OPS, CUSTOM_DVE_SPECS, _SUB_OPCODE_FOR_NAME
    from concourse.dve_spec import Spec, Src0, C0, C1, C2, One, sq, lower
    from concourse.dve_uop import DveOpSpec

    if "EXP_P4_ANT" in _SUB_OPCODE_FOR_NAME:
        return dve_ops.EXP_P4_ANT, dve_ops.EXP_SQ8_ANT

    u = Src0
    p4 = ((((u * C0) + C1) * u + C2) * u + One) * u + One  # c0=1/24 c1=1/6 c2=1/2
    spec_p4 = Spec(
        body=p4,
        reference=lambda in0, in1, s0, s1, imm2: (
            (((in0 * s0 + s1) * in0 + imm2) * in0 + 1.0) * in0 + 1.0
        ),
    )
    spec_sq8 = Spec(
        body=sq(sq(sq(Src0))),
        reference=lambda in0, in1, s0, s1, imm2: in0 ** 8,
    )

    def _mk(name, spec):
        opcode = max(_SUB_OPCODE_FOR_NAME.values()) + 1
        _SUB_OPCODE_FOR_NAME[name] = opcode
        shas = {}
        for ver in ("v3", "v4"):
            s = DveOpSpec(
                name=name, opcode=opcode, uops=lower(spec, ver=ver), rd1_en=False
            )
            shas[ver] = s.sha(ver)
        op = DveOp(name, spec, subdim=False, uops_sha=shas)
        OPS.append(op)
        CUSTOM_DVE_SPECS[name] = spec
        setattr(dve_ops, name, op)
        return op

    p4_op = _mk("EXP_P4_ANT", spec_p4)
    sq8_op = _mk("EXP_SQ8_ANT", spec_sq8)
    return p4_op, sq8_op


# ---------------------------------------------------------------- program build
_PROG_CACHE = {}


def build_program(use_dve_exp=True):
    key = ("prog", use_dve_exp)
    if key in _PROG_CACHE:
        return _PROG_CACHE[key]
    p4_op, sq8_op = _register_exp_ops()

    nc = bacc.Bacc("TRN2", target_bir_lowering=False, debug=False, num_devices=N_CORES)

    xT = nc.dram_tensor("xT", [D, N], F32, kind="ExternalInput").ap()
    wqkv = nc.dram_tensor("wqkv", [D, NM * 128], F32, kind="ExternalInput").ap()
    bqkv = nc.dram_tensor("bqkv", [NM, 128], F32, kind="ExternalInput").ap()
    wout = nc.dram_tensor("wout", [D, 192], F32, kind="ExternalInput").ap()
    bout = nc.dram_tensor("bout", [2, 128], F32, kind="ExternalInput").ap()
    y = nc.dram_tensor("y", [HPC * DH, N], F32, kind="ExternalOutput").ap()

    xT_r = xT.bitcast(F32R)
    wqkv_r = wqkv.bitcast(F32R)

    with tile.TileContext(nc, trace_sim=False) as tc:
        with (
            tc.tile_pool(name="consts", bufs=1) as consts,
            tc.tile_pool(name="persist", bufs=1) as persist,
            tc.tile_pool(name="otp", bufs=3) as otp,
            tc.tile_pool(name="spsum", bufs=2, space="PSUM") as spsum,
            tc.tile_pool(name="opsum", bufs=1, space="PSUM") as opsum,
            tc.tile_pool(name="dram", bufs=1, space="DRAM") as dram,
        ):
            # ---------------- constants
            ident_f = consts.tile([128, 128], F32)
            make_identity(nc, ident_f[:])
            ident = consts.tile([128, 128], BF16)
            nc.scalar.copy(ident[:], ident_f[:])

            ones_f = consts.tile([128, 64], F32)
            nc.vector.memset(ones_f[:], 1.0)
            ones1 = consts.tile([1, 64], F32R)  # K=1 stationary for recip broadcast
            nc.scalar.copy(ones1[:], ones_f[0:1, :])

            wq_sb = consts.tile([128, 6 * NM * 128], F32R)  # 6 f-chunks of [128,640]
            for fc in range(6):
                nc.sync.dma_start(
                    out=wq_sb[:, fc * NM * 128 : (fc + 1) * NM * 128],
                    in_=wqkv_r[fc * 128 : (fc + 1) * 128, :],
                )
            bq_sb = consts.tile([128, NM], F32)  # per-m-chunk bias columns
            for m in range(NM):
                nc.sync.dma_start(
                    out=bq_sb[:, m : m + 1],
                    in_=bqkv[m : m + 1, :].rearrange("a p -> p a"),
                )
            # phase-3 weights (loaded early, used late); fp32 staging + DVE
            # cast so the gpsimd queue stays empty for collectives
            wo_f = consts.tile([128, 6 * 192], F32)
            wo_sb = consts.tile([128, 6 * 192], BF16)  # w_out col-slice, 6 d-chunks
            for dc in range(6):
                nc.sync.dma_start(
                    out=wo_f[:, dc * 192 : (dc + 1) * 192],
                    in_=wout[dc * 128 : (dc + 1) * 128, :],
                )
                nc.vector.tensor_copy(
                    wo_sb[:, dc * 192 : (dc + 1) * 192],
                    wo_f[:, dc * 192 : (dc + 1) * 192],
                )
            bo_sb = consts.tile([128, 2], F32)
            for m in range(2):
                nc.sync.dma_start(
                    out=bo_sb[:, m : m + 1],
                    in_=bout[m : m + 1, :].rearrange("a p -> p a"),
                )

            # ---------------- persistent activations (bf16)
            qT01 = persist.tile([128, N], BF16)  # rows 0:64 head0 qT, 64:128 head1
            qT2 = persist.tile([128, N], BF16)   # head2 qT duplicated in both halves
            kT01 = persist.tile([128, N], BF16)  # scaled kT, heads 0/1
            kT2 = persist.tile([128, N], BF16)   # scaled kT head2, duplicated
            v_sb = persist.tile([128, 32 * VW], BF16)  # [tok128][v0|1|v1|1|v2|1]/j-tile

            # ones columns of v_sb (positions 64,129,194 of each 195 block)
            ones32 = consts.tile([128, 32], BF16)
            nc.vector.memset(ones32[:], 1.0)
            v_view = v_sb[:].rearrange("p (t c) -> p t c", c=VW)
            for col in (64, 129, 194):
                nc.vector.tensor_copy(
                    v_view[:, :, col : col + 1],
                    ones32[:].rearrange("p (a b) -> p a b", b=1),
                )

            # AllGather eighths: in [192, 512] -> out [768, 512] (rank-major rows)
            oT_q = [dram.tile([HPC * DH, IC], BF16, name=f"oT_q{i}") for i in range(8)]
            ag_q = [dram.tile([D, IC], BF16, name=f"ag_q{i}") for i in range(8)]

            with tc.tile_pool(name="work", bufs=2) as work, \
                 tc.tile_pool(name="ptp", bufs=3) as ptp:
                # ---------------- phase 1: qkv projection
                for tau in range(NTAU):
                    ts = slice(tau * IC, (tau + 1) * IC)
                    xts = []
                    for fc in range(6):
                        xt = work.tile([128, IC], F32R, tag="xt", bufs=7)
                        nc.sync.dma_start(
                            out=xt[:], in_=xT_r[fc * 128 : (fc + 1) * 128, ts]
                        )
                        xts.append(xt)
                    vt01 = work.tile([128, IC], BF16, tag="vt01")
                    vt2 = work.tile([64, IC], BF16, tag="vt2")
                    for m in range(NM):
                        pp = spsum.tile([128, 1024], F32, tag="sp", bufs=3)
                        for fc in range(6):
                            nc.tensor.matmul(
                                pp[:, 0:IC],
                                wq_sb[:, fc * NM * 128 + m * 128
                                      : fc * NM * 128 + (m + 1) * 128],
                                xts[fc][:],
                                start=(fc == 0),
                                stop=(fc == 5),
                            )
                        if m == 0:
                            nc.scalar.activation(
                                qT01[:, ts], pp[:, 0:IC], AF.Identity,
                                bias=bq_sb[:, 0:1],
                            )
                        elif m == 1:
                            nc.scalar.activation(
                                kT01[:, ts], pp[:, 0:IC], AF.Identity,
                                bias=bq_sb[:, 1:2],
                            )
                        elif m == 2:
                            nc.scalar.activation(
                                vt01[:], pp[:, 0:IC], AF.Identity,
                                bias=bq_sb[:, 2:3],
                            )
                        elif m == 3:
                            # [q2 | k2]: split halves, partition-aligned writes
                            nc.scalar.activation(
                                qT2[0:64, ts], pp[0:64, 0:IC], AF.Identity,
                                bias=bq_sb[0:64, 3:4],
                            )
                            nc.scalar.activation(
                                kT2[64:128, ts], pp[64:128, 0:IC], AF.Identity,
                                bias=bq_sb[64:128, 3:4],
                            )
                        else:
                            nc.scalar.activation(
                                vt2[:], pp[0:64, 0:IC], AF.Identity,
                                bias=bq_sb[0:64, 4:5],
                            )
                    # transpose v into token-major layout (DVE does the copies)
                    for t in range(4):
                        jt = 4 * tau + t
                        base = jt * VW
                        pv = spsum.tile([128, 2048], BF16, tag="sp", bufs=3)
                        nc.tensor.transpose(
                            pv[:, 0:128], vt01[:, t * 128 : (t + 1) * 128], ident[:]
                        )
                        nc.vector.tensor_copy(
                            v_sb[:, base : base + 130].rearrange(
                                "p (a b) -> p a b", b=65
                            )[:, :, 0:64],
                            pv[:, 0:128].rearrange("p (a b) -> p a b", b=64),
                        )
                        pv2 = spsum.tile([128, 2048], BF16, tag="sp", bufs=3)
                        nc.tensor.transpose(
                            pv2[:, 0:64],
                            vt2[:, t * 128 : (t + 1) * 128],
                            ident[0:64, 0:64],
                        )
                        nc.vector.tensor_copy(
                            v_sb[:, base + 130 : base + 194], pv2[:, 0:64]
                        )

                # duplicate head-2 q/k into the other row-half (SBUF->SBUF DMA)
                nc.sync.dma_start(out=qT2[64:128, :], in_=qT2[0:64, :])
                nc.sync.dma_start(out=kT2[0:64, :], in_=kT2[64:128, :])

                # ---------------- phase 2: attention
                exp_batch_idx = [0]

                def exp_batch(sp):
                    """exp(8u) on a [128,1024] psum batch -> bf16 SBUF tile."""
                    i = exp_batch_idx[0]
                    exp_batch_idx[0] += 1
                    pt = ptp.tile([128, 1024], BF16, tag="pt", bufs=5)
                    ck = i % 48
                    if use_dve_exp and ck < 45 and ck not in (29, 30, 31) \
                            and (ck % 10) in (0, 3, 6):
                        tmp = ptp.tile([128, 1024], F32, tag="exptmp", bufs=3)
                        nc.vector._custom_dve(
                            p4_op, out=tmp[:], in0=sp[:],
                            s0=1.0 / 24, s1=1.0 / 6, imm2=0.5,
                        )
                        nc.vector._custom_dve(sq8_op, out=pt[:], in0=tmp[:])
                    else:
                        nc.scalar.activation(pt[:], sp[:], AF.Exp, scale=8.0)
                    return pt

                def norm_recip(po):
                    """stage A: 1/l from psum row 64 (copied to partition 0
                    first — custom-DVE ops misread PSUM APs at base 64)."""
                    from concourse.dve_ops import (
                        RECIP_APPROX_FAST_CONSTS as RC,
                        RECIPROCAL_APPROX_FAST,
                    )
                    lrow = otp.tile([1, IC], F32, tag="lrow")
                    nc.vector.tensor_copy(lrow[:], po[64:65, :])
                    rr = otp.tile([1, IC], F32R, tag="rr")
                    with nc.allow_low_precision(reason="~18-bit recip of softmax denom"):
                        nc.vector._custom_dve(
                            RECIPROCAL_APPROX_FAST, out=rr[:], in0=lrow[:],
                            s0=RC["s0"], s1=RC["s1"], imm2=RC["imm2"],
                        )
                    return rr

                def norm_store(po, rr, ic, h):
                    """stage B: broadcast 1/l, scale, and DMA to oT_dram."""
                    pr = spsum.tile([128, 1024], F32, tag="sp", bufs=3)
                    nc.tensor.matmul(
                        pr[0:64, 0:IC], ones1[:], rr[:], start=True, stop=True
                    )
                    rb = otp.tile([64, IC], F32, tag="rb")
                    nc.scalar.copy(rb[:], pr[0:64, 0:IC])
                    ot = otp.tile([64, IC], BF16, tag="ot")
                    nc.vector.tensor_tensor(
                        out=ot[:], in0=po[0:64, :], in1=rb[:],
                        op=mybir.AluOpType.mult,
                    )
                    nc.sync.dma_start(
                        out=oT_q[ic][64 * h : 64 * h + 64, :], in_=ot[:]
                    )

                def project(qtr):
                    """output projection of gathered chunk qtr (col-parallel)."""
                    agr = ag_q[qtr][:]
                    ogs = []
                    for dc in range(6):
                        og = work.tile([128, IC], BF16, tag="og", bufs=12)
                        nc.sync.dma_start(
                            out=og[:], in_=agr[dc * 128 : (dc + 1) * 128, :]
                        )
                        ogs.append(og)
                    for ec, (elo, ew) in enumerate(((0, 128), (128, 64))):
                        py = spsum.tile([128, 1024], F32, tag="sp", bufs=3)
                        for dc in range(6):
                            nc.tensor.matmul(
                                py[0:ew, 0:IC],
                                wo_sb[:, dc * 192 + elo : dc * 192 + elo + ew],
                                ogs[dc][:],
                                start=(dc == 0), stop=(dc == 5),
                            )
                        ysb = work.tile([128, IC], F32, tag="ysb", bufs=3)
                        nc.scalar.activation(
                            ysb[0:ew, :], py[0:ew, 0:IC], AF.Identity,
                            bias=bo_sb[0:ew, ec : ec + 1],
                        )
                        nc.sync.dma_start(
                            out=y[elo : elo + ew, qtr * IC : (qtr + 1) * IC],
                            in_=ysb[0:ew, :],
                        )

                for ic in range(NIC):
                    isl = slice(ic * IC, (ic + 1) * IC)
                    # --- heads 0,1 (row-tiled pair over the same key tile)
                    po0 = opsum.tile([128, IC], F32, tag="po0", bufs=1)
                    po1 = opsum.tile([128, IC], F32, tag="po1", bufs=1)
                    for jc in range(32):
                        sp = spsum.tile([128, 1024], F32, tag="sp", bufs=3)
                        nc.tensor.matmul(
                            sp[:, 0:512],
                            kT01[0:64, jc * 128 : (jc + 1) * 128],
                            qT01[0:64, isl],
                            start=True, stop=True, tile_position=(0, 0),
                        )
                        nc.tensor.matmul(
                            sp[:, 512:1024],
                            kT01[64:128, jc * 128 : (jc + 1) * 128],
                            qT01[64:128, isl],
                            start=True, stop=True, tile_position=(64, 0),
                        )
                        pt = exp_batch(sp)
                        vbase = jc * VW
                        nc.tensor.matmul(
                            po0[0:65, :], v_sb[:, vbase : vbase + 65], pt[:, 0:512],
                            start=(jc == 0), stop=(jc == 31),
                        )
                        nc.tensor.matmul(
                            po1[0:65, :], v_sb[:, vbase + 65 : vbase + 130],
                            pt[:, 512:1024],
                            start=(jc == 0), stop=(jc == 31),
                        )
                    rr0 = norm_recip(po0)
                    rr1 = norm_recip(po1)
                    norm_store(po0, rr0, ic, 0)
                    norm_store(po1, rr1, ic, 1)
                    # --- head 2 (row-tiled pair over adjacent key tiles)
                    po2 = opsum.tile([128, IC], F32, tag="po0", bufs=1)
                    for t in range(16):
                        sp = spsum.tile([128, 1024], F32, tag="sp", bufs=3)
                        nc.tensor.matmul(
                            sp[:, 0:512],
                            kT2[0:64, (2 * t) * 128 : (2 * t + 1) * 128],
                            qT2[0:64, isl],
                            start=True, stop=True, tile_position=(0, 0),
                        )
                        nc.tensor.matmul(
                            sp[:, 512:1024],
                            kT2[64:128, (2 * t + 1) * 128 : (2 * t + 2) * 128],
                            qT2[64:128, isl],
                            start=True, stop=True, tile_position=(64, 0),
                        )
                        pt = exp_batch(sp)
                        for s in range(2):
                            jc = 2 * t + s
                            vbase = jc * VW
                            nc.tensor.matmul(
                                po2[0:65, :],
                                v_sb[:, vbase + 130 : vbase + 195],
                                pt[:, s * 512 : (s + 1) * 512],
                                start=(jc == 0), stop=(jc == 31),
                            )
                    rr2 = norm_recip(po2)
                    norm_store(po2, rr2, ic, 2)
                    nc.gpsimd.collective_compute(
                        "AllGather",
                        mybir.AluOpType.bypass,
                        replica_groups=[[0, 1, 2, 3], [4, 5, 6, 7]],
                        ins=[oT_q[ic][:]],
                        outs=[ag_q[ic][:]],
                    )
                # ---------------- phase 3: output projection (column-parallel)
                for qtr in range(NIC):
                    project(qtr)

    nc.compile()
    _PROG_CACHE[key] = nc
    return nc


# ---------------------------------------------------------------- host wrapper
def make_in_maps(x, w_qkv, b_qkv, w_out, b_out):
    """Build the 8 per-core input dicts from full inputs."""
    in_maps = []
    xTb = [np.ascontiguousarray(x[b].T) for b in range(B)]  # [768, 4096]
    kscale = np.float32(SCALE / 8.0)
    for c in range(N_CORES):
        b = c // 4
        hs = HPC * (c % 4)

        def sect(kind, h):  # q=0,k=1,v=2
            lo = kind * (H * DH) + h * DH
            return w_qkv[:, lo : lo + DH], b_qkv[lo : lo + DH]

        q0, bq0 = sect(0, hs); q1, bq1 = sect(0, hs + 1); q2, bq2 = sect(0, hs + 2)
        k0, bk0 = sect(1, hs); k1, bk1 = sect(1, hs + 1); k2, bk2 = sect(1, hs + 2)
        v0, bv0 = sect(2, hs); v1, bv1 = sect(2, hs + 1); v2, bv2 = sect(2, hs + 2)
        z = np.zeros_like(q2); bz = np.zeros_like(bq2)
        # m-chunks: [q0|q1], [k0|k1]*s, [v0|v1], [q2|k2]*s, [v2|0]
        cols = np.concatenate(
            [q0, q1, k0 * kscale, k1 * kscale, v0, v1,
             q2, k2 * kscale, v2, z], axis=1).astype(np.float32)
        bias = np.concatenate(
            [bq0, bq1, bk0 * kscale, bk1 * kscale, bv0, bv1,
             bq2, bk2 * kscale, bv2, bz]).astype(np.float32)
        q = c % 4
        bo = np.zeros((2, 128), np.float32)
        bo[0, :] = b_out[192 * q : 192 * q + 128]
        bo[1, :64] = b_out[192 * q + 128 : 192 * q + 192]
        in_maps.append({
            "xT": xTb[b],
            "wqkv": np.ascontiguousarray(cols),
            "bqkv": np.ascontiguousarray(bias.reshape(NM, 128)),
            "wout": np.ascontiguousarray(
                w_out[:, 192 * q : 192 * (q + 1)].astype(np.float32)),
            "bout": bo,
        })
    return in_maps


def assemble_output(results):
    out = np.empty((B, N, D), dtype=np.float32)
    for c in range(N_CORES):
        b = c // 4
        q = c % 4
        out[b, :, 192 * q : 192 * (q + 1)] = results[c]["y"].T
    return out


def kernel(x, w_qkv, b_qkv, w_out, b_out):
    from concourse.bass_utils import run_bass_kernel_spmd

    x = np.asarray(x, dtype=np.float32)
    nc = build_program()
    in_maps = make_in_maps(
        x, np.asarray(w_qkv, np.float32), np.asarray(b_qkv, np.float32),
        np.asarray(w_out, np.float32), np.asarray(b_out, np.float32))
    res = run_bass_kernel_spmd(nc, in_maps, core_ids=list(range(N_CORES)))
    return assemble_output(res.results)
